# revision 13
# baseline (speedup 1.0000x reference)
"""EdgeFtLayer GNN message-passing kernel for 8 TRN2 NeuronCores.

Strategy (edge-parallel, dst-partitioned):
  - Nodes padded to 50176 = 392 blocks of 128; core c owns 49 blocks.
  - Host sorts edges by dst, assigns each edge to the core owning its dst
    block, pads every (core, block) edge list to T_max tiles of 128 edges.
    Per-dst segment sums are then fully core-local.
  - Device: per-node projection table [50176, 384] =
    [A_s | T_s | P | A_d | T_d + b_T | P] built distributed (each core its
    slice) + AllGather.  Per edge tile: edge-feature matmul into PSUM,
    PSUM->SBUF copy, then two indirect-DMA gathers with CCE add fold the
    src-row and dst-row contributions directly into [z | u | eo] lanes.
    exp(prelu(z)) and p*u feed a one-hot matmul that segment-sums into the
    per-block accumulator; division + clamp produce new node features.

  Math notes:
   * segment-max subtraction is skipped: logits ~ N(0, 3.2), |l| <~ 12,
     exp is safe in f32 and num/denom is mathematically identical.
   * b_T is folded into the table's T_d column, so num/denom already
     includes + b_T; 0-degree nodes stay exactly 0 via denom clamp.
"""

import os
import time as _time
from contextlib import ExitStack

import numpy as np

import concourse.bass as bass
import concourse.mybir as mybir
from concourse import bacc
import concourse.tile as tile
from concourse.bass_utils import run_bass_kernel_spmd

N_NODES = 50000
N_EDGES = 800000
V_IN, V_OUT, E_IN, E_OUT = 128, 64, 64, 64
NCORES = 8
NB = 128                  # node block (PSUM partition dim)
BPC = 49                  # blocks per core
NPC = NB * BPC            # 6272 nodes per core
NBLK = NCORES * BPC       # 392 blocks
NPAD = NBLK * NB          # 50176 padded nodes
TBL_W = 384               # table cols [A_s|T_s|P|A_d|T_d'|P]
WPK = TBL_W + 192 + TBL_W  # wpack cols: Wnode | Wedge(pad) | brow(row0)

_kernel_cache = {}
LAST_RESULT = None
LAST_WALL_S = None


def _build(T_max: int, a_slope: float):
    f32 = mybir.dt.float32
    i32 = mybir.dt.int32
    cap = T_max * 128
    E_LOC = BPC * cap

    nc = bacc.Bacc()
    nfT = nc.declare_dram_parameter("nfT", [V_IN, NPC], f32, isOutput=False)
    wpack = nc.declare_dram_parameter("wpack", [V_IN, WPK], f32, isOutput=False)
    efT = nc.declare_dram_parameter("efT", [E_IN, E_LOC], f32, isOutput=False)
    srcb = nc.declare_dram_parameter("srcb", [BPC * 128, T_max], i32, isOutput=False)
    dstb = nc.declare_dram_parameter("dstb", [BPC * 128, T_max], i32, isOutput=False)
    relb = nc.declare_dram_parameter("relb", [BPC * 128, T_max], f32, isOutput=False)
    out_n = nc.declare_dram_parameter("out_n", [NPC, V_OUT], f32, isOutput=True)
    # p-major edge output: out_e[p, (b*T+t)*64 + c]
    out_e = nc.declare_dram_parameter("out_e", [128, BPC * T_max * 64], f32, isOutput=True)

    ag_in = nc.dram_tensor("ag_in", [NPC, TBL_W], f32)
    table = nc.dram_tensor("table", [NPAD, TBL_W], f32, addr_space="Shared")

    AF = mybir.ActivationFunctionType
    OP = mybir.AluOpType

    with tile.TileContext(nc) as tc, ExitStack() as ctx:
        constp = ctx.enter_context(tc.tile_pool(name="const", bufs=1))

        # constants (2 DMAs total)
        iota_i = constp.tile([128, 128], i32)
        iota_f = constp.tile([128, 128], f32)
        nc.gpsimd.iota(iota_i[:], pattern=[[1, 128]], base=0, channel_multiplier=0)
        nc.vector.tensor_copy(iota_f[:], iota_i[:])
        ones1 = constp.tile([1, 128], f32)
        nc.gpsimd.memset(ones1[:], 1.0)
        wpk_t = constp.tile([V_IN, WPK], f32)
        nc.sync.dma_start(wpk_t[:], wpack[:])
        nfT_t = constp.tile([V_IN, NPC], f32)
        nc.sync.dma_start(nfT_t[:], nfT[:])
        # DVE copies so matmuls wait on one engine sem, not extra DMA lanes
        # (walrus caps sync waits per matmul; Tile has no transitive elision)
        wnode_t = constp.tile([V_IN, TBL_W], f32)
        nc.vector.tensor_copy(wnode_t[:], wpk_t[:, 0:TBL_W])
        wedge_t = constp.tile([E_IN, 192], f32)
        nc.vector.tensor_copy(wedge_t[:], wpk_t[0:E_IN, TBL_W : TBL_W + 192])
        brow_t = constp.tile([1, TBL_W], f32)
        nc.vector.tensor_copy(brow_t[:], wpk_t[0:1, TBL_W + 192 : TBL_W + 192 + TBL_W])

        # ---- node phase: projection table slice -> single DMA -> AllGather
        # pools stay open for the whole kernel: SBUF/PSUM address reuse
        # across the phase boundary would add cross-phase WAR waits that
        # push matmuls over the ISA sync-wait limit
        nodep = ctx.enter_context(tc.tile_pool(name="nodep", bufs=1))
        psum_n = ctx.enter_context(tc.tile_pool(name="psum_n", bufs=2, space="PSUM"))
        if True:
            tbl_loc = nodep.tile([128, BPC * TBL_W], f32)
            for b in range(BPC):
                pt = psum_n.tile([128, TBL_W], f32, tag="pt_n")
                nc.tensor.matmul(
                    out=pt[:],
                    lhsT=nfT_t[:, b * 128 : (b + 1) * 128],
                    rhs=wnode_t[:],
                    start=True,
                    stop=False,
                )
                nc.tensor.matmul(
                    out=pt[:], lhsT=ones1[:], rhs=brow_t[:], start=False, stop=True
                )
                nc.vector.tensor_copy(tbl_loc[:, b * TBL_W : (b + 1) * TBL_W], pt[:])
            # one store: DRAM ap iterates [p, b, c] to match SBUF [p, (b c)]
            ag_view = ag_in[:].rearrange("(b p) c -> p b c", p=128)
            nc.sync.dma_start(ag_view, tbl_loc[:].rearrange("p (b c) -> p b c", b=BPC))

        nc.gpsimd.collective_compute(
            "AllGather",
            OP.bypass,
            replica_groups=[list(range(NCORES))],
            ins=[ag_in[:]],
            outs=[table[:]],
        )

        # ---- edge phase ----
        with (
            tc.tile_pool(name="eftp", bufs=2) as eftp,
            tc.tile_pool(name="idxp", bufs=2) as idxp,
            tc.tile_pool(name="Sp", bufs=2) as Sp,
            tc.tile_pool(name="PVp", bufs=2) as PVp,
            tc.tile_pool(name="ohp", bufs=4) as ohp,
            tc.tile_pool(name="scrp", bufs=4) as scrp,
            tc.tile_pool(name="epi", bufs=2) as epi,
            tc.tile_pool(name="psum_t", bufs=3, space="PSUM") as psum_t,
            tc.tile_pool(name="psum_b", bufs=2, space="PSUM") as psum_b,
        ):
            for b in range(BPC):
                eft_t = eftp.tile([E_IN, cap], f32, tag="eft")
                nc.sync.dma_start(eft_t[:], efT[:, b * cap : (b + 1) * cap])
                src_t = idxp.tile([128, T_max], i32, tag="src")
                nc.sync.dma_start(src_t[:], srcb[b * 128 : (b + 1) * 128, :])
                dst_t = idxp.tile([128, T_max], i32, tag="dst")
                nc.sync.dma_start(dst_t[:], dstb[b * 128 : (b + 1) * 128, :])
                rel_t = idxp.tile([128, T_max], f32, tag="rel")
                nc.sync.dma_start(rel_t[:], relb[b * 128 : (b + 1) * 128, :])

                S = Sp.tile([128, T_max * 192], f32, tag="S")
                PV = PVp.tile([128, T_max * 128], f32, tag="PV")

                # edge-feature projections -> S tiles
                for t in range(T_max):
                    pt = psum_t.tile([128, 192], f32, tag="pt")
                    nc.tensor.matmul(
                        out=pt[:],
                        lhsT=eft_t[:, t * 128 : (t + 1) * 128],
                        rhs=wedge_t[:],
                        start=True,
                        stop=True,
                    )
                    nc.vector.tensor_copy(S[:, t * 192 : (t + 1) * 192], pt[:])

                # fold src rows and dst rows into [z|u|eo] via CCE add.
                # HW honors only ONE index per partition per indirect DMA,
                # so gather per tile (128 rows/instruction).
                for t in range(T_max):
                    nc.gpsimd.indirect_dma_start(
                        out=S[:, t * 192 : (t + 1) * 192],
                        out_offset=None,
                        in_=table[:],
                        in_offset=bass.IndirectOffsetOnAxis(ap=src_t[:, t : t + 1], axis=0),
                        compute_op=OP.add,
                    )
                    nc.gpsimd.indirect_dma_start(
                        out=S[:, t * 192 : (t + 1) * 192],
                        out_offset=None,
                        in_=table[:],
                        in_offset=bass.IndirectOffsetOnAxis(ap=dst_t[:, t : t + 1], axis=0),
                        element_offset=192,
                        compute_op=OP.add,
                    )

                pb = psum_b.tile([128, 128], f32, tag="pb")
                for t in range(T_max):
                    z = S[:, t * 192 : t * 192 + 64]
                    u = S[:, t * 192 + 64 : t * 192 + 128]
                    p = PV[:, t * 128 : t * 128 + 64]
                    pu = PV[:, t * 128 + 64 : (t + 1) * 128]
                    # prelu(z) = max(z, a*z)   (a <= 1)
                    lr = scrp.tile([128, 64], f32, tag="lr")
                    nc.vector.tensor_scalar_mul(lr[:], z, a_slope)
                    nc.vector.tensor_tensor(out=lr[:], in0=lr[:], in1=z, op=OP.max)
                    pe_t = scrp.tile([128, 64], f32, tag="pe")
                    nc.scalar.activation(pe_t[:], lr[:], AF.Exp)
                    # keep PV single-writer (DVE) so the scatter matmul's
                    # wait list stays within the ISA limit
                    nc.vector.tensor_copy(p, pe_t[:])
                    nc.vector.tensor_tensor(out=pu, in0=pe_t[:], in1=u, op=OP.mult)
                    oh = ohp.tile([128, 128], f32, tag="oh")
                    nc.vector.tensor_tensor(
                        out=oh[:],
                        in0=iota_f[:],
                        in1=rel_t[:, t : t + 1].to_broadcast([128, 128]),
                        op=OP.is_equal,
                    )
                    nc.tensor.matmul(
                        out=pb[:],
                        lhsT=oh[:],
                        rhs=PV[:, t * 128 : (t + 1) * 128],
                        start=(t == 0),
                        stop=(t == T_max - 1),
                    )

                # edge outputs for the whole block in one DMA (p-major layout)
                eo_view = S[:].rearrange("p (t g) -> p t g", g=192)[:, :, 128:192]
                nc.sync.dma_start(
                    out_e[:, b * T_max * 64 : (b + 1) * T_max * 64].rearrange(
                        "p (t c) -> p t c", c=64
                    ),
                    eo_view,
                )

                # block epilogue: new_n = num / max(denom, tiny)
                dn = epi.tile([128, 64], f32, tag="dn")
                nc.vector.tensor_scalar_max(dn[:], pb[:, 0:64], 1e-30)
                rc = epi.tile([128, 64], f32, tag="rc")
                nc.vector.reciprocal(rc[:], dn[:])
                nn = epi.tile([128, 64], f32, tag="nn")
                nc.vector.tensor_tensor(out=nn[:], in0=pb[:, 64:128], in1=rc[:], op=OP.mult)
                nc.sync.dma_start(out_n[b * 128 : (b + 1) * 128, :], nn[:])

    nc.finalize()
    return nc, E_LOC, T_max


def _prep(src, dst, e_feat):
    """Sort edges by dst, partition into (core, block) buckets padded to
    T_max tiles of 128."""
    src = np.asarray(src).astype(np.int64)
    dst = np.asarray(dst).astype(np.int64)
    e_feat = np.asarray(e_feat, dtype=np.float32)

    order = np.argsort(dst, kind="stable")
    blk = (dst[order] // NB).astype(np.int64)
    counts = np.bincount(blk, minlength=NBLK)
    T_max = int(np.ceil(counts.max() / 128))
    cap = T_max * 128
    E_LOC = BPC * cap

    src_a = np.zeros((NCORES, E_LOC), np.int32)
    dst_a = np.zeros((NCORES, E_LOC), np.int32)
    rel_a = np.full((NCORES, E_LOC), -1.0, np.float32)
    ef_a = np.zeros((NCORES, E_LOC, E_IN), np.float32)
    eid_a = np.full((NCORES, E_LOC), -1, np.int64)

    starts = np.zeros(NBLK + 1, np.int64)
    starts[1:] = np.cumsum(counts)
    for b in range(NBLK):
        c, o = b // BPC, (b % BPC) * cap
        lo, hi = starts[b], starts[b + 1]
        n = hi - lo
        sl = order[lo:hi]
        src_a[c, o : o + n] = src[sl]
        dst_a[c, o : o + n] = dst[sl]
        rel_a[c, o : o + n] = (dst[sl] - b * NB).astype(np.float32)
        ef_a[c, o : o + n] = e_feat[sl]
        eid_a[c, o : o + n] = sl
        dst_a[c, o + n : o + cap] = b * NB  # valid row, zero one-hot

    per_core = []
    for c in range(NCORES):
        efT_c = np.ascontiguousarray(ef_a[c].T)  # [64, E_LOC]
        src_b = np.ascontiguousarray(
            src_a[c].reshape(BPC, T_max, 128).transpose(0, 2, 1).reshape(BPC * 128, T_max)
        )
        dst_b = np.ascontiguousarray(
            dst_a[c].reshape(BPC, T_max, 128).transpose(0, 2, 1).reshape(BPC * 128, T_max)
        )
        rel_b = np.ascontiguousarray(
            rel_a[c].reshape(BPC, T_max, 128).transpose(0, 2, 1).reshape(BPC * 128, T_max)
        )
        per_core.append((efT_c, src_b, dst_b, rel_b))
    return T_max, E_LOC, per_core, eid_a


def kernel(n_feat, e_feat, src, dst, W_a, W_T, b_T, W_e, W_ee, prelu_a):
    global LAST_RESULT, LAST_WALL_S
    n_feat = np.asarray(n_feat, dtype=np.float32)
    e_feat = np.asarray(e_feat, dtype=np.float32)
    W_a = np.asarray(W_a, dtype=np.float32)
    W_T = np.asarray(W_T, dtype=np.float32)
    b_T = np.asarray(b_T, dtype=np.float32)
    W_e = np.asarray(W_e, dtype=np.float32)
    W_ee = np.asarray(W_ee, dtype=np.float32)
    a_slope = float(np.asarray(prelu_a).ravel()[0])

    T_max, E_LOC, per_core, eid_a = _prep(src, dst, e_feat)

    key = (T_max, a_slope)
    if key not in _kernel_cache:
        _kernel_cache[key] = _build(T_max, a_slope)
    nc, _, _ = _kernel_cache[key]

    v, e = V_IN, E_IN
    Wnode = np.concatenate(
        [W_a[v + e :], W_T[v + e :], W_e, W_a[:v], W_T[:v], W_e], axis=1
    )  # [128, 384]
    Wedge_pad = np.zeros((V_IN, 192), np.float32)
    Wedge_pad[:e] = np.concatenate([W_a[v : v + e], W_T[v : v + e], W_ee], axis=1)
    brow_blk = np.zeros((V_IN, TBL_W), np.float32)
    brow_blk[0, 256:320] = b_T
    wpack = np.ascontiguousarray(
        np.concatenate([Wnode, Wedge_pad, brow_blk], axis=1)
    )  # [128, WPK]

    nf_pad = np.zeros((NPAD, V_IN), np.float32)
    nf_pad[:N_NODES] = n_feat

    in_maps = []
    for c in range(NCORES):
        efT_c, src_b, dst_b, rel_b = per_core[c]
        nfT_c = np.ascontiguousarray(nf_pad[c * NPC : (c + 1) * NPC].T)
        in_maps.append(
            {
                "nfT": nfT_c,
                "wpack": wpack,
                "efT": efT_c,
                "srcb": src_b,
                "dstb": dst_b,
                "relb": rel_b,
            }
        )

    t0 = _time.time()
    res = run_bass_kernel_spmd(nc, in_maps, list(range(NCORES)), trace=False)
    LAST_WALL_S = _time.time() - t0
    LAST_RESULT = res

    new_n = np.zeros((N_NODES, V_OUT), np.float32)
    new_e = np.zeros((N_EDGES, E_OUT), np.float32)
    for c in range(NCORES):
        on = np.asarray(res.results[c]["out_n"])
        oe = np.asarray(res.results[c]["out_e"])  # [128, BPC*T*64]
        lo, hi = c * NPC, min((c + 1) * NPC, N_NODES)
        new_n[lo:hi] = on[: hi - lo]
        # [p, bt, c] -> slot (bt*128+p)
        oe_slots = oe.reshape(128, BPC * T_max, 64).transpose(1, 0, 2).reshape(E_LOC, 64)
        valid = eid_a[c] >= 0
        new_e[eid_a[c][valid]] = oe_slots[valid]
    return new_n, new_e
